# revision 2
# baseline (speedup 1.0000x reference)
"""Distributed causal multi-head attention for TRN2, 8 NeuronCores.

Sharding: core c (0..7) handles batch c//4 and heads 4*(c%4)..4*(c%4)+3
(tensor-parallel over heads x data-parallel over batch). All matmuls
bf16 with fp32 PSUM accumulation (rel err ~5e-3 vs the f32 reference).

Per-core pipeline, ordered so the in-order PE queue never waits on an
ACT/DVE epilogue (everything is emission-scheduled by hand):
  1. Startup: x^T streams in 512-column quarters, split across the two
     DMA queues with W_Q/W_K (head 0) and W_V halves in front, so the
     first QT chain starts ~7us in and every later consumer's data
     lands before the PE reaches it.
  2. V[s,k] for all 4 heads (xt tiles stationary, heads packed in the
     free dim), interleaved with head 0's QT/KT chains as xt streams in.
  3. Per head: QT/KT[k,s] chunk chains (W stationary, host-pretransposed
     xT as rhs; b_Q/scale and b_K folded into the PSUM evacuation)
     interleaved with attention chunks. Inside a chunk the score matmul
     of tile i+1 is emitted BEFORE the zT matmul of tile i, so the
     in-order PE queue never waits on tile i's exp (ACT): per tile the
     PE does [ST_{i+1}, z_i] (426ns) while ACT does exp_{i+1} (427ns).
     The softmax denominator accumulates on the DVE (racc += pt); each
     chunk's epilogue (ones-block matmul broadcasts the denominator,
     DVE reciprocal+multiply normalize zT) is deferred into the next
     chunk's instruction stream, injected between ST_1 and z_0 where
     the PE would otherwise wait for exp_0.
  4. zT ships through AllGather over the 4-core batch group as it is
     produced: q-halves for heads 0-2, q-quarters for head 3, and the
     last chunk as 2x256-column pieces, so the collectives overlap
     attention and the final gathers don't queue on the CC engine.
  5. Output projection, d-sharded: each core's wo input holds only its
     512 W_O columns (out[all q, d_slice] = z_flat @ W_O[:, slice]),
     interleaved with head 3's tail as gathers land. The SPMD graph is
     identical on all cores; per-core behavior comes only from input
     data.

Host: shards/casts/transposes inputs, then adds the bias correction
b_O + sum_h b_V[h] @ W_O[h] (a constant row, because softmax rows sum
to 1) to the assembled output.
"""
import math
import os

import numpy as np
import ml_dtypes

import concourse.bacc as bacc
import concourse.mybir as mybir
from concourse import tile
from concourse.bass_utils import run_bass_kernel_spmd

BF16 = mybir.dt.bfloat16
F32 = mybir.dt.float32
NPBF16 = ml_dtypes.bfloat16

B = 2
SEQ = 2048
D_MODEL = 2048
N_HEADS = 16
D_HEAD = 128
HPC = 4              # heads per core
NCORES = 8
GROUPS = [[0, 1, 2, 3], [4, 5, 6, 7]]
NDT = D_MODEL // 128   # 16 d-model tiles
NST = SEQ // 128       # 16 seq tiles
NQC = SEQ // 512       # 4 q-chunks
QSL = SEQ // 4         # 512 per-core q-slice for output projection
SCALE = 1.0 / math.sqrt(D_HEAD)

LAST_EXEC_NS = None


def build_nc():
    nc = bacc.Bacc(None, num_devices=NCORES, debug=False)

    # xt is stored quarter-major on the host ([4*D_MODEL, 512]): quarter
    # qc holds x^T columns qc*512:(qc+1)*512, so each quarter is one
    # large contiguous DMA.
    xt_e = nc.declare_dram_parameter("xt", [NQC * D_MODEL, 512], BF16, isOutput=False)
    wq_e = nc.declare_dram_parameter("wq", [HPC * D_MODEL, D_HEAD], BF16, isOutput=False)
    wk_e = nc.declare_dram_parameter("wk", [HPC * D_MODEL, D_HEAD], BF16, isOutput=False)
    wv_e = nc.declare_dram_parameter("wv", [D_MODEL, HPC * D_HEAD], BF16, isOutput=False)
    wo_e = nc.declare_dram_parameter("wo", [N_HEADS * D_HEAD, QSL], BF16, isOutput=False)
    bq_e = nc.declare_dram_parameter("bq", [D_HEAD, HPC], F32, isOutput=False)
    bk_e = nc.declare_dram_parameter("bk", [D_HEAD, HPC], F32, isOutput=False)
    mk_e = nc.declare_dram_parameter("mk", [128, 128], BF16, isOutput=False)
    out_e = nc.declare_dram_parameter("out", [SEQ, QSL], F32, isOutput=True)

    # AllGather buffers: one per (local head, q-chunk). Quarters are
    # cheap on the CC engine and keep its queue from head-blocking the
    # final gathers that gate the output projection.
    # heads 0-2 gather per q-half (cheaper on the CC engine); head 3
    # per q-quarter so the output projection can chase its chunks
    agin = [[nc.dram_tensor(f"agin{h}_{hf}", [D_HEAD, SEQ // 2], BF16)
             for hf in range(2)] for h in range(3)]
    agout = [[nc.dram_tensor(f"agout{h}_{hf}", [4 * D_HEAD, SEQ // 2], BF16)
              for hf in range(2)] for h in range(3)]
    agin3 = [nc.dram_tensor(f"agin3_{j}", [D_HEAD, 512], BF16)
             for j in range(3)]
    agout3 = [nc.dram_tensor(f"agout3_{j}", [4 * D_HEAD, 512], BF16)
              for j in range(3)]
    # the last (h3, j3) gather is split into 2 x 256-column pieces so the
    # final output-projection chains pipeline with the collective
    agin_p = [nc.dram_tensor(f"aginp{p}", [D_HEAD, 256], BF16)
              for p in range(2)]
    agout_p = [nc.dram_tensor(f"agoutp{p}", [4 * D_HEAD, 256], BF16)
               for p in range(2)]

    with tile.TileContext(nc) as tc:
        with tc.tile_pool(name="persist", bufs=1) as pp, \
             tc.tile_pool(name="xtp", bufs=1) as xt_pool, \
             tc.tile_pool(name="qkp", bufs=2) as qk_pool, \
             tc.tile_pool(name="vp", bufs=HPC) as v_pool, \
             tc.tile_pool(name="wvp", bufs=1) as wv_pool, \
             tc.tile_pool(name="pt", bufs=5) as pt_pool, \
             tc.tile_pool(name="zz", bufs=3) as z_pool, \
             tc.tile_pool(name="wo", bufs=1) as wo_pool, \
             tc.tile_pool(name="zg", bufs=2) as zg_pool, \
             tc.tile_pool(name="os", bufs=1) as out_pool, \
             tc.tile_pool(name="ps1", bufs=2, space="PSUM") as ps1, \
             tc.tile_pool(name="ps_st", bufs=2, space="PSUM") as ps_st, \
             tc.tile_pool(name="ps_zt", bufs=2, space="PSUM") as ps_zt, \
             tc.tile_pool(name="ps_r", bufs=2, space="PSUM") as ps_r:
            ones_blk = pp.tile([128, 128], BF16, tag="ones_blk")
            nc.vector.memset(ones_blk[:], 1.0)

            # ---- startup DMA schedule -------------------------------------
            # Both queues carry the same byte schedule so neither lags:
            #   [wq0/2, xt_q0/2, wk0/2, (consts), wv/2, xt_q1/2, q2/2, q3/2]
            xt_sb = xt_pool.tile([128, NDT, SEQ], BF16, tag="xt")
            wq0 = qk_pool.tile([128, NDT, D_HEAD], BF16, tag="wq", name="wq0")
            wk0 = qk_pool.tile([128, NDT, D_HEAD], BF16, tag="wk", name="wk0")
            wv_sb = wv_pool.tile([128, NDT, HPC * D_HEAD], BF16, tag="wv")

            def dma_xt_quarter(qc):
                for half, eng in ((0, nc.sync), (1, nc.gpsimd)):
                    t0, t1 = half * 8, half * 8 + 8
                    eng.dma_start(
                        xt_sb[:, t0:t1, qc * 512:(qc + 1) * 512],
                        xt_e[qc * D_MODEL + t0 * 128:qc * D_MODEL + t1 * 128, :]
                        .rearrange("(t p) s -> p t s", p=128))

            def dma_w_halves(dst, src_e, row0):
                for half, eng in ((0, nc.sync), (1, nc.gpsimd)):
                    r0 = row0 + half * (D_MODEL // 2)
                    eng.dma_start(
                        dst[:, half * 8:half * 8 + 8, :],
                        src_e[r0:r0 + D_MODEL // 2, :]
                        .rearrange("(t p) k -> p t k", p=128))

            dma_w_halves(wq0, wq_e, 0)
            dma_xt_quarter(0)
            dma_w_halves(wk0, wk_e, 0)
            bq_sb = pp.tile([128, HPC], F32, tag="bq")
            nc.sync.dma_start(bq_sb[:], bq_e[:, :])
            bk_sb = pp.tile([128, HPC], F32, tag="bk")
            nc.sync.dma_start(bk_sb[:], bk_e[:, :])
            tri_sb = pp.tile([128, 128], BF16, tag="mk")
            nc.sync.dma_start(tri_sb[:], mk_e[:, :])
            for g4, eng in ((0, nc.sync), (1, nc.sync),
                            (2, nc.gpsimd), (3, nc.gpsimd)):
                eng.dma_start(
                    wv_sb[:, g4 * 4:(g4 + 1) * 4, :],
                    wv_e[g4 * 512:(g4 + 1) * 512, :]
                    .rearrange("(t p) k -> p t k", p=128))
            dma_xt_quarter(1)
            dma_xt_quarter(2)
            dma_xt_quarter(3)

            # ---- V for all 4 heads (packed rhs) --------------------------
            v_sb = [v_pool.tile([128, NST, D_HEAD], BF16, tag="v",
                                name=f"v{h}") for h in range(HPC)]
            wo_sb = wo_pool.tile([128, N_HEADS, QSL], BF16, tag="wo")

            def emit_v_chains(st0, st1):
                for st in range(st0, st1):
                    psum = ps1.tile([128, 512], F32, tag="ps1",
                                    name=f"pv{st}")
                    for dt in range(NDT):
                        nc.tensor.matmul(
                            psum[:],
                            xt_sb[:, dt, st * 128:(st + 1) * 128],
                            wv_sb[:, dt, :],
                            start=(dt == 0), stop=(dt == NDT - 1))
                    for h in range(HPC):
                        nc.scalar.copy(
                            v_sb[h][:, st, 0:D_HEAD],
                            psum[:, h * 128:(h + 1) * 128])

            # ---- helpers -------------------------------------------------
            def finalize_chunk(st):
                """Chunk epilogue: broadcast the softmax denominator with a
                ones-block matmul, then normalize and ship zT (everything
                except that one matmul runs off the PE)."""
                fh, fj, fz, fr = st
                rp = ps_r.tile([128, 512], F32, tag="r", name="rbc")
                nc.tensor.matmul(rp[:], ones_blk[:, :], fr[:],
                                 start=True, stop=True)
                rcp = z_pool.tile([128, 512], F32, tag="rcp")
                nc.vector.reciprocal(rcp[:], rp[:])
                zt = z_pool.tile([128, 512], BF16, tag="ztile")
                nc.vector.tensor_mul(zt[:], fz[:], rcp[:])
                if fh == 3 and fj == 3:
                    for p in range(2):
                        nc.sync.dma_start(
                            agin_p[p][:, :], zt[:, p * 256:(p + 1) * 256])
                        nc.gpsimd.collective_compute(
                            "AllGather",
                            mybir.AluOpType.bypass,
                            replica_groups=GROUPS,
                            ins=[agin_p[p].ap().opt()],
                            outs=[agout_p[p].ap().opt()],
                        )
                elif fh == 3:
                    nc.sync.dma_start(agin3[fj][:, :], zt[:])
                    nc.gpsimd.collective_compute(
                        "AllGather",
                        mybir.AluOpType.bypass,
                        replica_groups=GROUPS,
                        ins=[agin3[fj].ap().opt()],
                        outs=[agout3[fj].ap().opt()],
                    )
                else:
                    nc.sync.dma_start(
                        agin[fh][fj // 2][:, (fj % 2) * 512:(fj % 2 + 1) * 512],
                        zt[:])
                    if fj % 2 == 1:
                        nc.gpsimd.collective_compute(
                            "AllGather",
                            mybir.AluOpType.bypass,
                            replica_groups=GROUPS,
                            ins=[agin[fh][fj // 2].ap().opt()],
                            outs=[agout[fh][fj // 2].ap().opt()],
                        )

            def emit_zg(qg):
                """Load the gathered zT tiles for one 512-wide q-chunk.
                zg[:, h, r, :] = zT of global head 4*r + h."""
                zg = zg_pool.tile([128, HPC, 4, 512], BF16, tag="zg",
                                  name=f"zg{qg}")
                hf, qc = qg // 2, qg % 2
                for h in range(3):
                    nc.gpsimd.dma_start(
                        zg[:, h, :, :],
                        agout[h][hf][:, qc * 512:(qc + 1) * 512]
                        .rearrange("(r p) s -> p r s", p=128))
                if qg == 3:
                    for p in range(2):
                        nc.gpsimd.dma_start(
                            zg[:, 3, :, p * 256:(p + 1) * 256],
                            agout_p[p].ap()
                            .rearrange("(r p2) s -> p2 r s", p2=128))
                else:
                    nc.gpsimd.dma_start(
                        zg[:, 3, :, :],
                        agout3[qg].ap().rearrange("(r p) s -> p r s", p=128))
                return zg

            def emit_outproj(qg, zg, finalize_after_qi=None):
                osb = out_pool.tile([128, 4, QSL], F32, tag="os",
                                    name=f"os{qg}")
                for qi in range(4):
                    psum = ps1.tile([128, QSL], F32, tag="ps1",
                                    name=f"po{qg}_{qi}")
                    for tt in range(N_HEADS):
                        r, h = tt // HPC, tt % HPC
                        nc.tensor.matmul(
                            psum[:],
                            zg[:, h, r, qi * 128:(qi + 1) * 128],
                            wo_sb[:, tt, :],
                            start=(tt == 0), stop=(tt == N_HEADS - 1))
                    nc.scalar.copy(osb[:, qi, :], psum[:])
                    if qi == finalize_after_qi and pend[0] is not None:
                        finalize_chunk(pend[0])  # last (h3, j3) AllGather
                        pend[0] = None
                    if qi % 2 == 1:
                        nc.sync.dma_start(
                            out_e[qg * 512 + (qi - 1) * 128:
                                  qg * 512 + (qi + 1) * 128, :]
                            .rearrange("(t p) d -> p t d", p=128),
                            osb[:, qi - 1:qi + 1, :])

            # ---- per head: interleave QT/KT chunk chains with attention
            # chunks of the same head so no PE instruction ever waits on
            # an ACT/DVE epilogue; head 3 additionally interleaves the
            # output projection for q-chunks whose AllGathers have landed.
            pend = [None]
            zgs = {}

            def emit_qkt_chain(h, sc, wq_sb, wk_sb, qt_sb, kt_sb):
                for proj in range(2):
                    w_t = wq_sb if proj == 0 else wk_sb
                    dst = qt_sb if proj == 0 else kt_sb
                    psum = ps1.tile([128, 512], F32, tag="ps1",
                                    name=f"pq{h}_{sc}_{proj}")
                    for dt in range(NDT):
                        nc.tensor.matmul(
                            psum[:],
                            w_t[:, dt, :],
                            xt_sb[:, dt, sc * 512:(sc + 1) * 512],
                            start=(dt == 0), stop=(dt == NDT - 1))
                    if proj == 0:
                        nc.scalar.activation(
                            dst[:, sc * 512:(sc + 1) * 512], psum[:],
                            mybir.ActivationFunctionType.Identity,
                            bias=bq_sb[:, h:h + 1], scale=SCALE)
                    else:
                        nc.scalar.activation(
                            dst[:, sc * 512:(sc + 1) * 512], psum[:],
                            mybir.ActivationFunctionType.Identity,
                            bias=bk_sb[:, h:h + 1], scale=1.0)

            def emit_attn_chunk(h, j, qt_sb, kt_sb):
                """Software-pipelined: emit ST_{i+1} before z_i so the PE
                never waits on exp_i; the pending chunk's finalize matmul
                fills the PE slot where z_0 would wait on exp_0."""
                n_st = 4 * (j + 1)
                ztp = ps_zt.tile([128, 512], F32, tag="zt",
                                 name=f"zt{h}_{j}")
                racc = ps_r.tile([128, 512], F32, tag="r", name=f"ra{h}_{j}")

                def emit_st(i):
                    v = i - 4 * j
                    # causal: diagonal tile v touches only columns >= 128*v
                    c0 = 128 * v if v > 0 else 0
                    stp = ps_st.tile([128, 512], F32, tag="st")
                    nc.tensor.matmul(
                        stp[:, c0:],
                        kt_sb[:, i * 128:(i + 1) * 128],
                        qt_sb[:, j * 512 + c0:(j + 1) * 512],
                        start=True, stop=True)
                    pt = pt_pool.tile([128, 512], BF16, tag="pt")
                    nc.scalar.activation(
                        pt[:, c0:], stp[:, c0:],
                        mybir.ActivationFunctionType.Exp)
                    if v >= 0:
                        # mask only the 128-col triangle block at the diag
                        nc.vector.tensor_mul(
                            pt[:, c0:c0 + 128], pt[:, c0:c0 + 128],
                            tri_sb[:])
                    if i == 0:
                        nc.vector.tensor_copy(racc[:], pt[:])
                    else:
                        nc.vector.tensor_add(
                            racc[:, c0:], racc[:, c0:], pt[:, c0:])
                    return c0, pt

                def emit_z(i, c0, pt):
                    nc.tensor.matmul(
                        ztp[:, c0:], v_sb[h][:, i, :], pt[:, c0:],
                        start=(i == 0), stop=(i == n_st - 1))

                prev = emit_st(0)
                for i in range(1, n_st):
                    cur = emit_st(i)
                    if i == 1 and pend[0] is not None:
                        finalize_chunk(pend[0])
                        pend[0] = None
                    emit_z(i - 1, *prev)
                    prev = cur
                emit_z(n_st - 1, *prev)
                racc_sb = z_pool.tile([128, 512], BF16, tag="racc_sb")
                nc.scalar.copy(racc_sb[:], racc[:])
                pend[0] = (h, j, ztp, racc_sb)

            for h in range(HPC):
                if h == 1:
                    # W_O loads deferred past the startup DMA crunch
                    for g4, eng in ((0, nc.sync), (1, nc.gpsimd),
                                    (2, nc.sync), (3, nc.gpsimd)):
                        eng.dma_start(
                            wo_sb[:, g4 * 4:(g4 + 1) * 4, :],
                            wo_e[g4 * 512:(g4 + 1) * 512, :]
                            .rearrange("(t p) d -> p t d", p=128))
                if h == 0:
                    wq_sb, wk_sb = wq0, wk0
                else:
                    wq_sb = qk_pool.tile([128, NDT, D_HEAD], BF16, tag="wq",
                                         name=f"wq{h}")
                    nc.gpsimd.dma_start(
                        wq_sb[:],
                        wq_e[h * D_MODEL:(h + 1) * D_MODEL, :]
                        .rearrange("(t p) k -> p t k", p=128))
                    wk_sb = qk_pool.tile([128, NDT, D_HEAD], BF16, tag="wk",
                                         name=f"wk{h}")
                    nc.sync.dma_start(
                        wk_sb[:],
                        wk_e[h * D_MODEL:(h + 1) * D_MODEL, :]
                        .rearrange("(t p) k -> p t k", p=128))
                qt_sb = qk_pool.tile([128, SEQ], BF16, tag="qt", name=f"qt{h}")
                kt_sb = qk_pool.tile([128, SEQ], BF16, tag="kt", name=f"kt{h}")

                if h == 0:
                    # interleave the V chains into head 0's stream
                    emit_qkt_chain(h, 0, wq_sb, wk_sb, qt_sb, kt_sb)
                    emit_v_chains(0, 4)
                    emit_qkt_chain(h, 1, wq_sb, wk_sb, qt_sb, kt_sb)
                    emit_v_chains(4, 8)
                    emit_attn_chunk(h, 0, qt_sb, kt_sb)
                    emit_qkt_chain(h, 2, wq_sb, wk_sb, qt_sb, kt_sb)
                    emit_v_chains(8, 12)
                    emit_attn_chunk(h, 1, qt_sb, kt_sb)
                    emit_qkt_chain(h, 3, wq_sb, wk_sb, qt_sb, kt_sb)
                    emit_v_chains(12, 16)
                    emit_attn_chunk(h, 2, qt_sb, kt_sb)
                    emit_attn_chunk(h, 3, qt_sb, kt_sb)
                    continue
                emit_qkt_chain(h, 0, wq_sb, wk_sb, qt_sb, kt_sb)
                emit_qkt_chain(h, 1, wq_sb, wk_sb, qt_sb, kt_sb)
                emit_attn_chunk(h, 0, qt_sb, kt_sb)
                emit_qkt_chain(h, 2, wq_sb, wk_sb, qt_sb, kt_sb)
                emit_attn_chunk(h, 1, qt_sb, kt_sb)
                emit_qkt_chain(h, 3, wq_sb, wk_sb, qt_sb, kt_sb)
                emit_attn_chunk(h, 2, qt_sb, kt_sb)
                if h == 3:
                    # q-chunk 0's quarter-AG landed during chunk 1; its
                    # zg loads were queued behind that AG on gpsimd.
                    zgs[0] = emit_zg(0)
                    emit_outproj(0, zgs[0])
                emit_attn_chunk(h, 3, qt_sb, kt_sb)
                if h == 3:
                    zgs[1] = emit_zg(1)

            emit_outproj(1, zgs[1], finalize_after_qi=0)
            zgs[2] = emit_zg(2)
            zgs[3] = emit_zg(3)
            emit_outproj(2, zgs[2])
            emit_outproj(3, zgs[3])
    nc.finalize()
    return nc


def _build_tri():
    """tri[r, c] = 1 if key offset r <= query offset c (within the
    128x128 diagonal block; the same triangle serves every diagonal)."""
    r = np.arange(128)[:, None]
    c = np.arange(128)[None, :]
    return (c >= r).astype(NPBF16)


_NC_CACHE = None


def kernel(normalized_resid_pre, W_Q, b_Q, W_K, b_K, W_V, b_V, W_O, b_O):
    global LAST_EXEC_NS, _NC_CACHE
    x = np.asarray(normalized_resid_pre, dtype=np.float32)
    W_Q = np.asarray(W_Q, np.float32); b_Q = np.asarray(b_Q, np.float32)
    W_K = np.asarray(W_K, np.float32); b_K = np.asarray(b_K, np.float32)
    W_V = np.asarray(W_V, np.float32); b_V = np.asarray(b_V, np.float32)
    W_O = np.asarray(W_O, np.float32); b_O = np.asarray(b_O, np.float32)

    tri_m = _build_tri()
    wo_flat = W_O.reshape(N_HEADS * D_HEAD, D_MODEL)
    # quarter-major x^T: rows qc*D_MODEL..(qc+1)*D_MODEL hold columns
    # qc*512:(qc+1)*512 of x[b].T
    xt = []
    for b in range(B):
        xT = np.ascontiguousarray(x[b].T)
        xq = np.ascontiguousarray(
            xT.reshape(D_MODEL, NQC, 512).transpose(1, 0, 2)
            .reshape(NQC * D_MODEL, 512)).astype(NPBF16)
        xt.append(xq)

    in_maps = []
    for c in range(NCORES):
        beta, g = c // 4, c % 4
        hs = slice(HPC * g, HPC * g + HPC)
        wq_m = np.ascontiguousarray(
            W_Q[hs].reshape(HPC * D_MODEL, D_HEAD)).astype(NPBF16)
        wk_m = np.ascontiguousarray(
            W_K[hs].reshape(HPC * D_MODEL, D_HEAD)).astype(NPBF16)
        wv_m = np.ascontiguousarray(
            W_V[hs].transpose(1, 0, 2).reshape(D_MODEL, HPC * D_HEAD)).astype(NPBF16)
        wo_m = np.ascontiguousarray(
            wo_flat[:, QSL * g:QSL * (g + 1)]).astype(NPBF16)
        bq_m = np.ascontiguousarray((b_Q[hs] * SCALE).T).astype(np.float32)
        bk_m = np.ascontiguousarray(b_K[hs].T).astype(np.float32)
        in_maps.append({
            "xt": xt[beta], "wq": wq_m, "wk": wk_m, "wv": wv_m,
            "wo": wo_m, "bq": bq_m, "bk": bk_m, "mk": tri_m,
        })

    if _NC_CACHE is None:
        _NC_CACHE = build_nc()
    nc = _NC_CACHE

    trace = False
    if os.environ.get("BASS_KERNEL_TRACE") == "1":
        try:
            from antenv.axon_hooks import get_axon_ntff_profile_hook
            trace = get_axon_ntff_profile_hook() is not None
        except ImportError:
            trace = False

    res = run_bass_kernel_spmd(nc, in_maps, core_ids=list(range(NCORES)),
                               trace=trace)
    LAST_EXEC_NS = res.exec_time_ns

    # bias correction: softmax rows sum to 1 -> b_V contributes a constant
    # row through W_O; b_O is a plain add.
    corr = b_O + np.einsum("hk,hkd->d", b_V, W_O)

    out = np.empty((B, SEQ, D_MODEL), dtype=np.float32)
    for c in range(NCORES):
        beta, g = c // 4, c % 4
        out[beta, :, QSL * g:QSL * (g + 1)] = (
            res.results[c]["out"] + corr[QSL * g:QSL * (g + 1)])
    return out


# revision 4
# speedup vs baseline: 1.0125x; 1.0125x over previous
"""Distributed causal multi-head attention for TRN2, 8 NeuronCores.

Sharding: core c (0..7) handles batch c//4 and heads 4*(c%4)..4*(c%4)+3
(tensor-parallel over heads x data-parallel over batch). All matmuls
bf16 with fp32 PSUM accumulation (rel err ~5e-3 vs the f32 reference).

Per-core pipeline, ordered so the in-order PE queue never waits on an
ACT/DVE epilogue (everything is emission-scheduled by hand):
  1. Startup: x^T streams in 512-column quarters, split across the two
     DMA queues with W_Q/W_K (head 0) and W_V halves in front, so the
     first QT chain starts ~7us in and every later consumer's data
     lands before the PE reaches it.
  2. V[s,k] for all 4 heads (xt tiles stationary, heads packed in the
     free dim), interleaved with head 0's QT/KT chains as xt streams in.
  3. Per head: QT/KT[k,s] chunk chains (W stationary, host-pretransposed
     xT as rhs; b_Q/scale and b_K folded into the PSUM evacuation)
     interleaved with attention chunks. Inside a chunk the score matmul
     of tile i+1 is emitted BEFORE the zT matmul of tile i, so the
     in-order PE queue never waits on tile i's exp (ACT): per tile the
     PE does [ST_{i+1}, z_i] (426ns) while ACT does exp_{i+1} (427ns).
     The softmax denominator accumulates on the DVE (racc += pt); each
     chunk's epilogue (ones-block matmul broadcasts the denominator,
     DVE reciprocal+multiply normalize zT) is deferred into the next
     chunk's instruction stream, injected between ST_1 and z_0 where
     the PE would otherwise wait for exp_0.
  4. zT ships through AllGather over the 4-core batch group as it is
     produced: q-halves for heads 0-2, q-quarters for head 3, and the
     last chunk as 2x256-column pieces, so the collectives overlap
     attention and the final gathers don't queue on the CC engine.
  5. Output projection, d-sharded: each core's wo input holds only its
     512 W_O columns (out[all q, d_slice] = z_flat @ W_O[:, slice]),
     interleaved with head 3's tail as gathers land. The SPMD graph is
     identical on all cores; per-core behavior comes only from input
     data.

Host: shards/casts/transposes inputs, then adds the bias correction
b_O + sum_h b_V[h] @ W_O[h] (a constant row, because softmax rows sum
to 1) to the assembled output.
"""
import math
import os

import numpy as np
import ml_dtypes

import concourse.bacc as bacc
import concourse.mybir as mybir
from concourse import tile
from concourse.bass_utils import run_bass_kernel_spmd

BF16 = mybir.dt.bfloat16
F32 = mybir.dt.float32
NPBF16 = ml_dtypes.bfloat16

B = 2
SEQ = 2048
D_MODEL = 2048
N_HEADS = 16
D_HEAD = 128
HPC = 4              # heads per core
NCORES = 8
GROUPS = [[0, 1, 2, 3], [4, 5, 6, 7]]
NDT = D_MODEL // 128   # 16 d-model tiles
NST = SEQ // 128       # 16 seq tiles
NQC = SEQ // 512       # 4 q-chunks
QSL = SEQ // 4         # 512 per-core q-slice for output projection
SCALE = 1.0 / math.sqrt(D_HEAD)

LAST_EXEC_NS = None


def build_nc():
    nc = bacc.Bacc(None, num_devices=NCORES, debug=False)

    # xt is stored quarter-major on the host ([4*D_MODEL, 512]): quarter
    # qc holds x^T columns qc*512:(qc+1)*512, so each quarter is one
    # large contiguous DMA.
    xt_e = nc.declare_dram_parameter("xt", [NQC * D_MODEL, 512], BF16, isOutput=False)
    wq_e = nc.declare_dram_parameter("wq", [HPC * D_MODEL, D_HEAD], BF16, isOutput=False)
    wk_e = nc.declare_dram_parameter("wk", [HPC * D_MODEL, D_HEAD], BF16, isOutput=False)
    wv_e = nc.declare_dram_parameter("wv", [D_MODEL, HPC * D_HEAD], BF16, isOutput=False)
    wo_e = nc.declare_dram_parameter("wo", [N_HEADS * D_HEAD, QSL], BF16, isOutput=False)
    bq_e = nc.declare_dram_parameter("bq", [D_HEAD, HPC], F32, isOutput=False)
    bk_e = nc.declare_dram_parameter("bk", [D_HEAD, HPC], F32, isOutput=False)
    mk_e = nc.declare_dram_parameter("mk", [128, 128], BF16, isOutput=False)
    out_e = nc.declare_dram_parameter("out", [SEQ, QSL], F32, isOutput=True)

    # AllGather buffers: one per (local head, q-chunk). Quarters are
    # cheap on the CC engine and keep its queue from head-blocking the
    # final gathers that gate the output projection.
    # heads 0-2 gather per q-half (cheaper on the CC engine); head 3
    # per q-quarter so the output projection can chase its chunks
    agin = [[nc.dram_tensor(f"agin{h}_{hf}", [D_HEAD, SEQ // 2], BF16)
             for hf in range(2)] for h in range(3)]
    agout = [[nc.dram_tensor(f"agout{h}_{hf}", [4 * D_HEAD, SEQ // 2], BF16)
              for hf in range(2)] for h in range(3)]
    agin3 = [nc.dram_tensor(f"agin3_{j}", [D_HEAD, 512], BF16)
             for j in range(3)]
    agout3 = [nc.dram_tensor(f"agout3_{j}", [4 * D_HEAD, 512], BF16)
              for j in range(3)]
    # the last (h3, j3) gather is split into 2 x 256-column pieces so the
    # final output-projection chains pipeline with the collective
    agin_p = [nc.dram_tensor(f"aginp{p}", [D_HEAD, 256], BF16)
              for p in range(2)]
    agout_p = [nc.dram_tensor(f"agoutp{p}", [4 * D_HEAD, 256], BF16)
               for p in range(2)]

    with tile.TileContext(nc) as tc:
        with tc.tile_pool(name="persist", bufs=1) as pp, \
             tc.tile_pool(name="xtp", bufs=1) as xt_pool, \
             tc.tile_pool(name="qkp", bufs=2) as qk_pool, \
             tc.tile_pool(name="vp", bufs=HPC) as v_pool, \
             tc.tile_pool(name="wvp", bufs=1) as wv_pool, \
             tc.tile_pool(name="pt", bufs=5) as pt_pool, \
             tc.tile_pool(name="zz", bufs=3) as z_pool, \
             tc.tile_pool(name="wo", bufs=1) as wo_pool, \
             tc.tile_pool(name="zg", bufs=2) as zg_pool, \
             tc.tile_pool(name="os", bufs=1) as out_pool, \
             tc.tile_pool(name="ps1", bufs=2, space="PSUM") as ps1, \
             tc.tile_pool(name="ps_st", bufs=2, space="PSUM") as ps_st, \
             tc.tile_pool(name="ps_zt", bufs=2, space="PSUM") as ps_zt, \
             tc.tile_pool(name="ps_r", bufs=2, space="PSUM") as ps_r:
            ones_blk = pp.tile([128, 128], BF16, tag="ones_blk")
            nc.vector.memset(ones_blk[:], 1.0)

            # ---- startup DMA schedule -------------------------------------
            # Both queues carry the same byte schedule so neither lags:
            #   [wq0/2, xt_q0/2, wk0/2, (consts), wv/2, xt_q1/2, q2/2, q3/2]
            xt_sb = xt_pool.tile([128, NDT, SEQ], BF16, tag="xt")
            wq0 = qk_pool.tile([128, NDT, D_HEAD], BF16, tag="wq", name="wq0")
            wk0 = qk_pool.tile([128, NDT, D_HEAD], BF16, tag="wk", name="wk0")
            wv_sb = wv_pool.tile([128, NDT, HPC * D_HEAD], BF16, tag="wv")

            def dma_xt_quarter(qc):
                for half, eng in ((0, nc.sync), (1, nc.gpsimd)):
                    t0, t1 = half * 8, half * 8 + 8
                    eng.dma_start(
                        xt_sb[:, t0:t1, qc * 512:(qc + 1) * 512],
                        xt_e[qc * D_MODEL + t0 * 128:qc * D_MODEL + t1 * 128, :]
                        .rearrange("(t p) s -> p t s", p=128))

            def dma_w_halves(dst, src_e, row0):
                for half, eng in ((0, nc.sync), (1, nc.gpsimd)):
                    r0 = row0 + half * (D_MODEL // 2)
                    eng.dma_start(
                        dst[:, half * 8:half * 8 + 8, :],
                        src_e[r0:r0 + D_MODEL // 2, :]
                        .rearrange("(t p) k -> p t k", p=128))

            dma_w_halves(wq0, wq_e, 0)
            dma_xt_quarter(0)
            dma_w_halves(wk0, wk_e, 0)
            bq_sb = pp.tile([128, HPC], F32, tag="bq")
            nc.sync.dma_start(bq_sb[:], bq_e[:, :])
            bk_sb = pp.tile([128, HPC], F32, tag="bk")
            nc.sync.dma_start(bk_sb[:], bk_e[:, :])
            tri_sb = pp.tile([128, 128], BF16, tag="mk")
            nc.sync.dma_start(tri_sb[:], mk_e[:, :])
            for g4, eng in ((0, nc.sync), (1, nc.sync),
                            (2, nc.gpsimd), (3, nc.gpsimd)):
                eng.dma_start(
                    wv_sb[:, g4 * 4:(g4 + 1) * 4, :],
                    wv_e[g4 * 512:(g4 + 1) * 512, :]
                    .rearrange("(t p) k -> p t k", p=128))
            dma_xt_quarter(1)
            dma_xt_quarter(2)
            dma_xt_quarter(3)

            # ---- V for all 4 heads (packed rhs) --------------------------
            v_sb = [v_pool.tile([128, NST, D_HEAD], BF16, tag="v",
                                name=f"v{h}") for h in range(HPC)]
            wo_sb = wo_pool.tile([128, N_HEADS, QSL], BF16, tag="wo")

            def emit_v_chains(st0, st1):
                for st in range(st0, st1):
                    psum = ps1.tile([128, 512], F32, tag="ps1",
                                    name=f"pv{st}")
                    for dt in range(NDT):
                        nc.tensor.matmul(
                            psum[:],
                            xt_sb[:, dt, st * 128:(st + 1) * 128],
                            wv_sb[:, dt, :],
                            start=(dt == 0), stop=(dt == NDT - 1))
                    for h in range(HPC):
                        nc.scalar.copy(
                            v_sb[h][:, st, 0:D_HEAD],
                            psum[:, h * 128:(h + 1) * 128])

            # ---- helpers -------------------------------------------------
            def finalize_chunk(st):
                """Chunk epilogue: broadcast the softmax denominator with a
                ones-block matmul, then normalize and ship zT (everything
                except that one matmul runs off the PE)."""
                fh, fj, fz, fr = st
                rp = ps_r.tile([128, 512], F32, tag="r", name="rbc")
                nc.tensor.matmul(rp[:], ones_blk[:, :], fr[:],
                                 start=True, stop=True)
                rcp = z_pool.tile([128, 512], F32, tag="rcp")
                nc.vector.reciprocal(rcp[:], rp[:])
                zt = z_pool.tile([128, 512], BF16, tag="ztile")
                nc.vector.tensor_mul(zt[:], fz[:], rcp[:])
                if fh == 3 and fj == 3:
                    for p in range(2):
                        nc.sync.dma_start(
                            agin_p[p][:, :], zt[:, p * 256:(p + 1) * 256])
                        nc.gpsimd.collective_compute(
                            "AllGather",
                            mybir.AluOpType.bypass,
                            replica_groups=GROUPS,
                            ins=[agin_p[p].ap().opt()],
                            outs=[agout_p[p].ap().opt()],
                        )
                elif fh == 3:
                    nc.sync.dma_start(agin3[fj][:, :], zt[:])
                    nc.gpsimd.collective_compute(
                        "AllGather",
                        mybir.AluOpType.bypass,
                        replica_groups=GROUPS,
                        ins=[agin3[fj].ap().opt()],
                        outs=[agout3[fj].ap().opt()],
                    )
                else:
                    nc.sync.dma_start(
                        agin[fh][fj // 2][:, (fj % 2) * 512:(fj % 2 + 1) * 512],
                        zt[:])
                    if fj % 2 == 1:
                        nc.gpsimd.collective_compute(
                            "AllGather",
                            mybir.AluOpType.bypass,
                            replica_groups=GROUPS,
                            ins=[agin[fh][fj // 2].ap().opt()],
                            outs=[agout[fh][fj // 2].ap().opt()],
                        )

            def emit_zg(qg):
                """Load the gathered zT tiles for one 512-wide q-chunk.
                zg[:, h, r, :] = zT of global head 4*r + h. For the tail
                chunks (qg >= 2) the long-landed gathers load via the sync
                queue so they don't sit behind collectives on gpsimd."""
                zg = zg_pool.tile([128, HPC, 4, 512], BF16, tag="zg",
                                  name=f"zg{qg}")
                hf, qc = qg // 2, qg % 2
                eng = nc.sync if qg >= 2 else nc.gpsimd
                for h in range(3):
                    eng.dma_start(
                        zg[:, h, :, :],
                        agout[h][hf][:, qc * 512:(qc + 1) * 512]
                        .rearrange("(r p) s -> p r s", p=128))
                if qg == 3:
                    for p in range(2):
                        nc.gpsimd.dma_start(
                            zg[:, 3, :, p * 256:(p + 1) * 256],
                            agout_p[p].ap()
                            .rearrange("(r p2) s -> p2 r s", p2=128))
                else:
                    eng.dma_start(
                        zg[:, 3, :, :],
                        agout3[qg].ap().rearrange("(r p) s -> p r s", p=128))
                return zg

            def emit_outproj(qg, zg, finalize_after_qi=None):
                osb = out_pool.tile([128, 4, QSL], F32, tag="os",
                                    name=f"os{qg}")
                # local head 3's gather lands last; accumulate its tiles at
                # the end of each chain so the chain can start while the
                # final gather's zg loads are still streaming in.
                tts = [tt for tt in range(N_HEADS) if tt % HPC != 3] + \
                      [tt for tt in range(N_HEADS) if tt % HPC == 3]
                for qi in range(4):
                    psum = ps1.tile([128, QSL], F32, tag="ps1",
                                    name=f"po{qg}_{qi}")
                    for k, tt in enumerate(tts):
                        r, h = tt // HPC, tt % HPC
                        nc.tensor.matmul(
                            psum[:],
                            zg[:, h, r, qi * 128:(qi + 1) * 128],
                            wo_sb[:, tt, :],
                            start=(k == 0), stop=(k == N_HEADS - 1))
                    nc.scalar.copy(osb[:, qi, :], psum[:])
                    if qi == finalize_after_qi and pend[0] is not None:
                        finalize_chunk(pend[0])  # last (h3, j3) AllGather
                        pend[0] = None
                    if qi % 2 == 1:
                        nc.sync.dma_start(
                            out_e[qg * 512 + (qi - 1) * 128:
                                  qg * 512 + (qi + 1) * 128, :]
                            .rearrange("(t p) d -> p t d", p=128),
                            osb[:, qi - 1:qi + 1, :])

            # ---- per head: interleave QT/KT chunk chains with attention
            # chunks of the same head so no PE instruction ever waits on
            # an ACT/DVE epilogue; head 3 additionally interleaves the
            # output projection for q-chunks whose AllGathers have landed.
            pend = [None]
            zgs = {}

            def emit_qkt_chain(h, sc, wq_sb, wk_sb, qt_sb, kt_sb):
                for proj in range(2):
                    w_t = wq_sb if proj == 0 else wk_sb
                    dst = qt_sb if proj == 0 else kt_sb
                    psum = ps1.tile([128, 512], F32, tag="ps1",
                                    name=f"pq{h}_{sc}_{proj}")
                    for dt in range(NDT):
                        nc.tensor.matmul(
                            psum[:],
                            w_t[:, dt, :],
                            xt_sb[:, dt, sc * 512:(sc + 1) * 512],
                            start=(dt == 0), stop=(dt == NDT - 1))
                    if proj == 0:
                        nc.scalar.activation(
                            dst[:, sc * 512:(sc + 1) * 512], psum[:],
                            mybir.ActivationFunctionType.Identity,
                            bias=bq_sb[:, h:h + 1], scale=SCALE)
                    else:
                        nc.scalar.activation(
                            dst[:, sc * 512:(sc + 1) * 512], psum[:],
                            mybir.ActivationFunctionType.Identity,
                            bias=bk_sb[:, h:h + 1], scale=1.0)

            def emit_attn_chunk(h, j, qt_sb, kt_sb):
                """Software-pipelined: emit ST_{i+1} before z_i so the PE
                never waits on exp_i; the pending chunk's finalize matmul
                fills the PE slot where z_0 would wait on exp_0."""
                n_st = 4 * (j + 1)
                ztp = ps_zt.tile([128, 512], F32, tag="zt",
                                 name=f"zt{h}_{j}")
                racc = ps_r.tile([128, 512], F32, tag="r", name=f"ra{h}_{j}")

                def emit_st(i):
                    v = i - 4 * j
                    # causal: diagonal tile v touches only columns >= 128*v
                    c0 = 128 * v if v > 0 else 0
                    stp = ps_st.tile([128, 512], F32, tag="st")
                    nc.tensor.matmul(
                        stp[:, c0:],
                        kt_sb[:, i * 128:(i + 1) * 128],
                        qt_sb[:, j * 512 + c0:(j + 1) * 512],
                        start=True, stop=True)
                    pt = pt_pool.tile([128, 512], BF16, tag="pt")
                    nc.scalar.activation(
                        pt[:, c0:], stp[:, c0:],
                        mybir.ActivationFunctionType.Exp)
                    if v >= 0:
                        # mask only the 128-col triangle block at the diag
                        nc.vector.tensor_mul(
                            pt[:, c0:c0 + 128], pt[:, c0:c0 + 128],
                            tri_sb[:])
                    if i == 0:
                        nc.vector.tensor_copy(racc[:], pt[:])
                    else:
                        nc.vector.tensor_add(
                            racc[:, c0:], racc[:, c0:], pt[:, c0:])
                    return c0, pt

                def emit_z(i, c0, pt):
                    nc.tensor.matmul(
                        ztp[:, c0:], v_sb[h][:, i, :], pt[:, c0:],
                        start=(i == 0), stop=(i == n_st - 1))

                prev = emit_st(0)
                for i in range(1, n_st):
                    cur = emit_st(i)
                    if i == 1 and pend[0] is not None:
                        finalize_chunk(pend[0])
                        pend[0] = None
                    emit_z(i - 1, *prev)
                    prev = cur
                emit_z(n_st - 1, *prev)
                racc_sb = z_pool.tile([128, 512], BF16, tag="racc_sb")
                nc.scalar.copy(racc_sb[:], racc[:])
                pend[0] = (h, j, ztp, racc_sb)

            for h in range(HPC):
                if h == 1:
                    # W_O loads deferred past the startup DMA crunch
                    for g4, eng in ((0, nc.sync), (1, nc.gpsimd),
                                    (2, nc.sync), (3, nc.gpsimd)):
                        eng.dma_start(
                            wo_sb[:, g4 * 4:(g4 + 1) * 4, :],
                            wo_e[g4 * 512:(g4 + 1) * 512, :]
                            .rearrange("(t p) d -> p t d", p=128))
                if h == 0:
                    wq_sb, wk_sb = wq0, wk0
                else:
                    wq_sb = qk_pool.tile([128, NDT, D_HEAD], BF16, tag="wq",
                                         name=f"wq{h}")
                    nc.gpsimd.dma_start(
                        wq_sb[:],
                        wq_e[h * D_MODEL:(h + 1) * D_MODEL, :]
                        .rearrange("(t p) k -> p t k", p=128))
                    wk_sb = qk_pool.tile([128, NDT, D_HEAD], BF16, tag="wk",
                                         name=f"wk{h}")
                    nc.sync.dma_start(
                        wk_sb[:],
                        wk_e[h * D_MODEL:(h + 1) * D_MODEL, :]
                        .rearrange("(t p) k -> p t k", p=128))
                qt_sb = qk_pool.tile([128, SEQ], BF16, tag="qt", name=f"qt{h}")
                kt_sb = qk_pool.tile([128, SEQ], BF16, tag="kt", name=f"kt{h}")

                if h == 0:
                    # interleave the V chains into head 0's stream
                    emit_qkt_chain(h, 0, wq_sb, wk_sb, qt_sb, kt_sb)
                    emit_v_chains(0, 4)
                    emit_qkt_chain(h, 1, wq_sb, wk_sb, qt_sb, kt_sb)
                    emit_v_chains(4, 8)
                    emit_attn_chunk(h, 0, qt_sb, kt_sb)
                    emit_qkt_chain(h, 2, wq_sb, wk_sb, qt_sb, kt_sb)
                    emit_v_chains(8, 12)
                    emit_attn_chunk(h, 1, qt_sb, kt_sb)
                    emit_qkt_chain(h, 3, wq_sb, wk_sb, qt_sb, kt_sb)
                    emit_v_chains(12, 16)
                    emit_attn_chunk(h, 2, qt_sb, kt_sb)
                    emit_attn_chunk(h, 3, qt_sb, kt_sb)
                    continue
                emit_qkt_chain(h, 0, wq_sb, wk_sb, qt_sb, kt_sb)
                emit_qkt_chain(h, 1, wq_sb, wk_sb, qt_sb, kt_sb)
                emit_attn_chunk(h, 0, qt_sb, kt_sb)
                emit_qkt_chain(h, 2, wq_sb, wk_sb, qt_sb, kt_sb)
                emit_attn_chunk(h, 1, qt_sb, kt_sb)
                emit_qkt_chain(h, 3, wq_sb, wk_sb, qt_sb, kt_sb)
                emit_attn_chunk(h, 2, qt_sb, kt_sb)
                if h == 3:
                    # q-chunk 0's quarter-AG landed during chunk 1; its
                    # zg loads were queued behind that AG on gpsimd.
                    zgs[0] = emit_zg(0)
                    emit_outproj(0, zgs[0])
                emit_attn_chunk(h, 3, qt_sb, kt_sb)
                if h == 3:
                    zgs[1] = emit_zg(1)

            emit_outproj(1, zgs[1], finalize_after_qi=0)
            zgs[2] = emit_zg(2)
            zgs[3] = emit_zg(3)
            emit_outproj(2, zgs[2])
            emit_outproj(3, zgs[3])
    nc.finalize()
    return nc


def _build_tri():
    """tri[r, c] = 1 if key offset r <= query offset c (within the
    128x128 diagonal block; the same triangle serves every diagonal)."""
    r = np.arange(128)[:, None]
    c = np.arange(128)[None, :]
    return (c >= r).astype(NPBF16)


_NC_CACHE = None


def kernel(normalized_resid_pre, W_Q, b_Q, W_K, b_K, W_V, b_V, W_O, b_O):
    global LAST_EXEC_NS, _NC_CACHE
    x = np.asarray(normalized_resid_pre, dtype=np.float32)
    W_Q = np.asarray(W_Q, np.float32); b_Q = np.asarray(b_Q, np.float32)
    W_K = np.asarray(W_K, np.float32); b_K = np.asarray(b_K, np.float32)
    W_V = np.asarray(W_V, np.float32); b_V = np.asarray(b_V, np.float32)
    W_O = np.asarray(W_O, np.float32); b_O = np.asarray(b_O, np.float32)

    tri_m = _build_tri()
    wo_flat = W_O.reshape(N_HEADS * D_HEAD, D_MODEL)
    # quarter-major x^T: rows qc*D_MODEL..(qc+1)*D_MODEL hold columns
    # qc*512:(qc+1)*512 of x[b].T
    xt = []
    for b in range(B):
        xT = np.ascontiguousarray(x[b].T)
        xq = np.ascontiguousarray(
            xT.reshape(D_MODEL, NQC, 512).transpose(1, 0, 2)
            .reshape(NQC * D_MODEL, 512)).astype(NPBF16)
        xt.append(xq)

    in_maps = []
    for c in range(NCORES):
        beta, g = c // 4, c % 4
        hs = slice(HPC * g, HPC * g + HPC)
        wq_m = np.ascontiguousarray(
            W_Q[hs].reshape(HPC * D_MODEL, D_HEAD)).astype(NPBF16)
        wk_m = np.ascontiguousarray(
            W_K[hs].reshape(HPC * D_MODEL, D_HEAD)).astype(NPBF16)
        wv_m = np.ascontiguousarray(
            W_V[hs].transpose(1, 0, 2).reshape(D_MODEL, HPC * D_HEAD)).astype(NPBF16)
        wo_m = np.ascontiguousarray(
            wo_flat[:, QSL * g:QSL * (g + 1)]).astype(NPBF16)
        bq_m = np.ascontiguousarray((b_Q[hs] * SCALE).T).astype(np.float32)
        bk_m = np.ascontiguousarray(b_K[hs].T).astype(np.float32)
        in_maps.append({
            "xt": xt[beta], "wq": wq_m, "wk": wk_m, "wv": wv_m,
            "wo": wo_m, "bq": bq_m, "bk": bk_m, "mk": tri_m,
        })

    if _NC_CACHE is None:
        _NC_CACHE = build_nc()
    nc = _NC_CACHE

    trace = False
    if os.environ.get("BASS_KERNEL_TRACE") == "1":
        try:
            from antenv.axon_hooks import get_axon_ntff_profile_hook
            trace = get_axon_ntff_profile_hook() is not None
        except ImportError:
            trace = False

    res = run_bass_kernel_spmd(nc, in_maps, core_ids=list(range(NCORES)),
                               trace=trace)
    LAST_EXEC_NS = res.exec_time_ns

    # bias correction: softmax rows sum to 1 -> b_V contributes a constant
    # row through W_O; b_O is a plain add.
    corr = b_O + np.einsum("hk,hkd->d", b_V, W_O)

    out = np.empty((B, SEQ, D_MODEL), dtype=np.float32)
    for c in range(NCORES):
        beta, g = c // 4, c % 4
        out[beta, :, QSL * g:QSL * (g + 1)] = (
            res.results[c]["out"] + corr[QSL * g:QSL * (g + 1)])
    return out
